# revision 16
# baseline (speedup 1.0000x reference)
"""Trainium2 Bass kernel for CustomGRUModel.

Reference computation (per batch row):
    gx = x @ W                       # [T, 3H] input projections
    per step t:
        gh_zr = h @ U[:, :2H]
        z = sigmoid(gxz + ghz + bz)
        r = sigmoid(gxr + ghr + br)
        n = tanh(gxn + (r*h) @ U[:, 2H:] + bn)
        h = z*h + (1-z)*n
    y = h_last @ Wd + bd

Sharding: data-parallel over batch, 32 rows per core on 8 cores. Weights
replicated. No collectives.

Per-core design (v5, fp16):
  - All matmul operands in fp16 (U, W, x, h, rh). fp32 would be lowered as
    TWO HW passes per matmul with a full-rate LDWEIGHTS each; 16-bit is one
    pass with 2x fast-weight-load, and at N=32 the weight loads fully hide
    behind the streaming (measured ~32ns per matmul). fp16 over bf16: same
    speed, 8x the mantissa precision (all values here are well in range).
  - Layout "features on partitions": hT [H=512, B=32] as TWO persistent
    SBUF tiles h_lo/h_hi [128, (k=2)x(b=32)] (split so consumers of each
    half release as soon as its updater finishes — Tile dependency
    tracking is per-tile). Gate matmuls keep U tiles [128,128] stationary,
    stream hT chunks (N=32), k-outer so each h/rh half is consumed as soon
    as it is ready; outputs land [3H, B] in PSUM so elementwise runs on
    full 128 partitions.
  - gx = x@W is computed in chunks of TC=4 steps directly INTO the
    recurrence PSUM banks; gate matmuls accumulate on top (start=False),
    so there are no per-step gx adds at all. Three per-gate PSUM tiles
    (z/r/n) of exactly one 2KB bank each, layout [128, (t=TC)(m=4)(b=32)]
    fp32, double-buffered (6 banks) + 1 bank x-transpose staging.
    PSUM "pending zero" discipline: the chronologically-first matmul into
    each bank uses start=True (marks the whole bank pending-zero: first
    touch overwrites, later touches accumulate), everything after uses
    start=False; the chronologically-last one uses stop=True.
  - Per-step critical chain: r-matmuls -> sigmoid(r) -> r*h (2 halves) ->
    n-matmuls (k-outer) -> tanh -> (1-z)*n -> h halves. z-path (sigmoid,
    1-z via VectorE, z*h on GpSimd) overlaps the n window. The x@W
    precompute for the next chunk is spliced in after the n-matmuls to
    fill TensorE during the tail.
"""

import os

import numpy as np

B, T, D, H = 256, 512, 256, 512
NCORES = 8
BL = B // NCORES  # 32 batch rows per core
TC = 4  # timestep chunk for the gx precompute (one PSUM bank per gate)
KH = H // 128  # 4 k-tiles over H
KD = D // 128  # 2 k-tiles over D
M3H = 3 * H // 128  # 12 m-tiles over 3H

_CACHE = {}


def _build(t_run, with_bias):
    from contextlib import ExitStack

    import concourse.bacc as bacc
    import concourse.bass as bass
    import concourse.tile as tile
    from concourse import masks, mybir

    dt = mybir.dt
    f32 = dt.float32
    f16 = dt.float16
    AF = mybir.ActivationFunctionType

    nchunk = t_run // TC

    nc = bacc.Bacc(
        "TRN2", target_bir_lowering=False, debug=False, num_devices=NCORES
    )
    x_d = nc.dram_tensor("x", [BL, T, D], f32, kind="ExternalInput")
    w_d = nc.dram_tensor("W", [D, 3 * H], f32, kind="ExternalInput")
    u_d = nc.dram_tensor("U", [H, 3 * H], f32, kind="ExternalInput")
    b_d = nc.dram_tensor("b", [3 * H], f32, kind="ExternalInput")
    wd_d = nc.dram_tensor("Wd", [H, 1], f32, kind="ExternalInput")
    bd_d = nc.dram_tensor("bd", [1], f32, kind="ExternalInput")
    y_d = nc.dram_tensor("y", [BL, 1], f32, kind="ExternalOutput")

    # chunked view of x: [chunk, tc, b, d]
    x_view = x_d.rearrange("b (c t) d -> c t b d", t=TC)

    with tile.TileContext(nc) as tc, ExitStack() as ctx:
        const = ctx.enter_context(tc.tile_pool(name="const", bufs=1))
        xin_pool = ctx.enter_context(tc.tile_pool(name="xin", bufs=4))
        xt_pool = ctx.enter_context(tc.tile_pool(name="xt", bufs=2))
        sb_pool = ctx.enter_context(tc.tile_pool(name="sb", bufs=3))
        # one PSUM pool per gate group; each tile is exactly one 2KB bank
        gz_psum = ctx.enter_context(
            tc.tile_pool(name="gzp", bufs=2, space=bass.MemorySpace.PSUM)
        )
        gr_psum = ctx.enter_context(
            tc.tile_pool(name="grp", bufs=2, space=bass.MemorySpace.PSUM)
        )
        gn_psum = ctx.enter_context(
            tc.tile_pool(name="gnp", bufs=2, space=bass.MemorySpace.PSUM)
        )
        xt_psum = ctx.enter_context(
            tc.tile_pool(name="xtp", bufs=2, space=bass.MemorySpace.PSUM)
        )

        # ---- constants (load fp32, cast to fp16 working copies) ----
        stage = const.tile([128, 3 * H], f32)
        u_sb = const.tile([128, KH, 3 * H], f16)
        for k in range(KH):
            nc.sync.dma_start(stage[:], u_d[k * 128 : (k + 1) * 128, :])
            nc.vector.tensor_copy(u_sb[:, k, :], stage[:])
        w_sb = const.tile([128, KD, 3 * H], f16)
        for k in range(KD):
            nc.sync.dma_start(stage[:], w_d[k * 128 : (k + 1) * 128, :])
            nc.vector.tensor_copy(w_sb[:, k, :], stage[:])

        b_sb = const.tile([128, M3H], f32)
        nc.sync.dma_start(b_sb[:], b_d.rearrange("(m p) -> p m", p=128))

        wd_stage = const.tile([128, KH], f32)
        nc.sync.dma_start(wd_stage[:], wd_d.rearrange("(k p) o -> p (k o)", p=128))
        wd_sb = const.tile([128, KH], f16)
        nc.vector.tensor_copy(wd_sb[:], wd_stage[:])
        bd_sb = const.tile([1, 1], f32)
        nc.sync.dma_start(bd_sb[0:1, :], bd_d.rearrange("(o u) -> o u", u=1))
        bd_f16 = const.tile([1, 1], f16)
        nc.vector.tensor_copy(bd_f16[0:1, :], bd_sb[0:1, :])
        ident = const.tile([128, 128], f32)
        masks.make_identity(nc, ident[:])
        ones_sb = const.tile([1, BL], f16)
        nc.gpsimd.memset(ones_sb[0:1, :], 1.0)

        # persistent hidden state hT, split into halves (k-chunks 0-1 / 2-3)
        h_half = [
            const.tile([128, 2 * BL], f16, name=f"h{i}") for i in range(2)
        ]
        nc.gpsimd.memset(h_half[0][:], 0.0)
        nc.gpsimd.memset(h_half[1][:], 0.0)

        def h_slice(k):
            return h_half[k // 2][:, (k % 2) * BL : (k % 2 + 1) * BL]

        warm_ps = xt_psum.tile([128, KD, 128], f32, name="warm", tag="xtp")
        nc.tensor.transpose(warm_ps[:, 0, :], ident[:], ident[:])

        gx_tiles = {}

        def make_units(c):
            """Emit-thunks for precomputing gx chunk c (TC steps) into PSUM.

            Per-gate tiles (z: m=0..3, r: 4..7, n: 8..11), layout
            [128, (t=TC)(mm=4)(b=32)] fp32 = one 2KB bank each, so the
            per-step gate slice [:, t, :, :] is 128 contiguous columns for
            the ScalarE reads. First matmul into each tile (mm=0, kd=0)
            uses start=True; all later ones start=False.
            """
            parts = (
                gz_psum.tile([128, TC, 4, BL], f32, name="gz", tag="gzp"),
                gr_psum.tile([128, TC, 4, BL], f32, name="gr", tag="grp"),
                gn_psum.tile([128, TC, 4, BL], f32, name="gn", tag="gnp"),
            )
            gx_tiles[c] = parts
            xin = xin_pool.tile([128, D], f32, name="xin", tag="xin")
            xt_ps = xt_psum.tile([128, KD, TC * BL], f32, name="xtp", tag="xtp")
            xt_sb = xt_pool.tile([128, KD, TC * BL], f16, name="xt", tag="xt")
            units = []

            def load():
                nc.sync.dma_start(xin[:], x_view[c])

            def tr(kd):
                nc.tensor.transpose(
                    xt_ps[:, kd, :], xin[:, 128 * kd : 128 * (kd + 1)], ident[:]
                )

            def evict(kd):
                nc.vector.tensor_copy(xt_sb[:, kd, :], xt_ps[:, kd, :])

            def mm(m):
                part = parts[m // 4]
                mm_i = m % 4
                for kd in range(KD):
                    nc.tensor.matmul(
                        part[:, :, mm_i, :],
                        w_sb[:, kd, m * 128 : (m + 1) * 128],
                        xt_sb[:, kd, :],
                        start=(kd == 0 and mm_i == 0),
                        stop=False,
                        skip_group_check=True,
                    )

            units.append(load)
            for kd in range(KD):
                units.append(lambda kd=kd: tr(kd))
            for kd in range(KD):
                units.append(lambda kd=kd: evict(kd))
            for m in range(M3H):
                units.append(lambda m=m: mm(m))
            return units

        def emit_step(c, j, mid_units, dist):
            """One GRU step; gates accumulate into gx chunk tiles at t=j.

            The r-gate pre-activations for THIS step were already
            accumulated into gr during the previous step (distributed
            h-update: U_r@h = U_r@(z*h) + U_r@((1-z)*n); step 0 needs
            nothing since h0=0). This step emits the distributed r
            matmuls for the NEXT step into `dist` = (gr_tile, t, is_last)
            or None at the end of the sequence.

            mid_units: precompute emit-thunks spliced in after the n
            matmuls so they fill TensorE during the tanh tail.
            """
            gz_t, gr_t, gn_t = gx_tiles[c]
            last = j == TC - 1  # last step of chunk: emit stop=True per bank

            r_sb = sb_pool.tile([128, KH * BL], f16, name="r", tag="r")
            rh_half = [
                sb_pool.tile([128, 2 * BL], f16, name=f"rh{i}", tag=f"rh{i}")
                for i in range(2)
            ]
            z_sb = sb_pool.tile([128, KH * BL], f16, name="z", tag="z")
            zc_sb = sb_pool.tile([128, KH * BL], f16, name="zc", tag="zc")
            zh_sb = sb_pool.tile([128, KH * BL], f16, name="zh", tag="zh")
            n_sb = sb_pool.tile([128, KH * BL], f16, name="n", tag="n")
            zcn_sb = sb_pool.tile([128, KH * BL], f16, name="zcn", tag="zcn")

            # z gates (m=0..3), h-based, k-outer
            for k in range(KH):
                for mm_i in range(4):
                    nc.tensor.matmul(
                        gz_t[:, j, mm_i, :],
                        u_sb[:, k, mm_i * 128 : (mm_i + 1) * 128],
                        h_slice(k),
                        start=False,
                        stop=(last and k == KH - 1 and mm_i == 3),
                        skip_group_check=True,
                    )

            if with_bias:
                for i in range(4):
                    nc.scalar.activation(
                        r_sb[:, i * BL : (i + 1) * BL],
                        gr_t[:, j, i, :],
                        AF.Sigmoid,
                        bias=b_sb[:, 4 + i : 5 + i],
                    )
            else:
                nc.scalar.activation(
                    r_sb[:].rearrange("p (m b) -> p m b", m=4),
                    gr_t[:, j, :, :],
                    AF.Sigmoid,
                )
            for i in range(2):
                nc.vector.tensor_mul(
                    rh_half[i][:], r_sb[:, i * 2 * BL : (i + 1) * 2 * BL],
                    h_half[i][:],
                )

            if with_bias:
                for i in range(4):
                    nc.scalar.activation(
                        z_sb[:, i * BL : (i + 1) * BL],
                        gz_t[:, j, i, :],
                        AF.Sigmoid,
                        bias=b_sb[:, i : i + 1],
                    )
            else:
                nc.scalar.activation(
                    z_sb[:].rearrange("p (m b) -> p m b", m=4),
                    gz_t[:, j, :, :],
                    AF.Sigmoid,
                )
            nc.vector.tensor_scalar(
                zc_sb[:], z_sb[:], -1.0, 1.0,
                mybir.AluOpType.mult, mybir.AluOpType.add,
            )
            for i in range(2):
                nc.gpsimd.tensor_mul(
                    zh_sb[:, i * 2 * BL : (i + 1) * 2 * BL],
                    z_sb[:, i * 2 * BL : (i + 1) * 2 * BL],
                    h_half[i][:],
                )

            # n gates (m=8..11), k-outer: consume rh halves as they appear
            for k in range(KH):
                for mm_i in range(4):
                    nc.tensor.matmul(
                        gn_t[:, j, mm_i, :],
                        u_sb[:, k, (8 + mm_i) * 128 : (9 + mm_i) * 128],
                        rh_half[k // 2][:, (k % 2) * BL : (k % 2 + 1) * BL],
                        start=False,
                        stop=(last and k == KH - 1 and mm_i == 3),
                        skip_group_check=True,
                    )

            # distributed r matmuls for the NEXT step, z*h part (available
            # mid-window, runs during the tanh tail)
            if dist is not None:
                ngr_t, nj, nlast = dist
                for k in range(KH):
                    for mm_i in range(4):
                        nc.tensor.matmul(
                            ngr_t[:, nj, mm_i, :],
                            u_sb[:, k, (4 + mm_i) * 128 : (5 + mm_i) * 128],
                            zh_sb[:, k * BL : (k + 1) * BL],
                            start=False,
                            stop=False,
                            skip_group_check=True,
                        )

            # precompute filler: runs on TensorE during the tanh/update tail
            for u in mid_units:
                u()

            if with_bias:
                for i in range(4):
                    nc.scalar.activation(
                        n_sb[:, i * BL : (i + 1) * BL],
                        gn_t[:, j, i, :],
                        AF.Tanh,
                        bias=b_sb[:, 8 + i : 9 + i],
                    )
            else:
                nc.scalar.activation(
                    n_sb[:].rearrange("p (m b) -> p m b", m=4),
                    gn_t[:, j, :, :],
                    AF.Tanh,
                )
            nc.vector.tensor_mul(zcn_sb[:], zc_sb[:], n_sb[:])

            # distributed r matmuls for the NEXT step, (1-z)*n part — this
            # is the only piece of the next r pre-activation on the chain.
            if dist is not None:
                ngr_t, nj, nlast = dist
                for k in range(KH):
                    for mm_i in range(4):
                        nc.tensor.matmul(
                            ngr_t[:, nj, mm_i, :],
                            u_sb[:, k, (4 + mm_i) * 128 : (5 + mm_i) * 128],
                            zcn_sb[:, k * BL : (k + 1) * BL],
                            start=False,
                            stop=(nlast and k == KH - 1 and mm_i == 3),
                            skip_group_check=True,
                        )

            # h = z*h + (1-z)*n (off the critical chain now; feeds the
            # next step's z matmuls, r*h and z*h)
            for i in range(2):
                nc.vector.tensor_add(
                    h_half[i][:],
                    zh_sb[:, i * 2 * BL : (i + 1) * 2 * BL],
                    zcn_sb[:, i * 2 * BL : (i + 1) * 2 * BL],
                )

        # ---- main emission ----
        for u in make_units(0):
            u()
        for c in range(nchunk):
            pend = make_units(c + 1) if c + 1 < nchunk else []
            done = 0
            for j in range(TC):
                g = c * TC + j
                if g + 1 < t_run:
                    nc_, njj = divmod(g + 1, TC)
                    dist = (gx_tiles[nc_][1], njj, njj == TC - 1)
                else:
                    dist = None
                want = (len(pend) * (j + 1) + TC - 1) // TC
                mid = []
                while done < min(want, len(pend)):
                    mid.append(pend[done])
                    done += 1
                emit_step(c, j, mid, dist)
            while done < len(pend):
                pend[done]()
                done += 1

        # final dense head: y = h @ Wd + bd
        out_ps = xt_psum.tile([128, KD, 128], f32, name="outp", tag="xtp")
        for k in range(KH):
            nc.tensor.matmul(
                out_ps[0:BL, 0, 0:1],
                h_slice(k),
                wd_sb[:, k : k + 1],
                start=(k == 0),
                stop=False,
            )
        nc.tensor.matmul(
            out_ps[0:BL, 0, 0:1],
            ones_sb[0:1, :],
            bd_f16[0:1, :],
            start=False,
            stop=True,
        )
        y_sb = sb_pool.tile([BL, 1], f32, name="y", tag="y")
        nc.vector.tensor_copy(y_sb[:], out_ps[0:BL, 0, 0:1])
        nc.sync.dma_start(y_d[:], y_sb[:])

    nc.compile()
    return nc


def kernel(x, W, U, b, Wd, bd):
    from concourse.bass_utils import run_bass_kernel_spmd

    t_run = int(os.environ.get("GRU_T_RUN", T))

    x = np.ascontiguousarray(np.asarray(x, dtype=np.float32))
    W = np.ascontiguousarray(np.asarray(W, dtype=np.float32))
    U = np.ascontiguousarray(np.asarray(U, dtype=np.float32))
    b = np.ascontiguousarray(np.asarray(b, dtype=np.float32))
    Wd = np.ascontiguousarray(np.asarray(Wd, dtype=np.float32))
    bd = np.ascontiguousarray(np.asarray(bd, dtype=np.float32))

    with_bias = bool(np.any(b != 0.0))
    key = (t_run, with_bias)
    if key not in _CACHE:
        _CACHE[key] = _build(t_run, with_bias)
    nc = _CACHE[key]

    in_maps = [
        {
            "x": np.ascontiguousarray(x[i * BL : (i + 1) * BL]),
            "W": W,
            "U": U,
            "b": b,
            "Wd": Wd,
            "bd": bd,
        }
        for i in range(NCORES)
    ]
    res = run_bass_kernel_spmd(
        nc,
        in_maps,
        core_ids=list(range(NCORES)),
        trace=os.environ.get("GRU_TRACE", "0") == "1",
    )
    out = np.concatenate([r["y"] for r in res.results], axis=0)
    if res.exec_time_ns is not None:
        print(f"HW exec time: {res.exec_time_ns} ns")
    return out


# revision 18
# speedup vs baseline: 8.5308x; 8.5308x over previous
"""Trainium2 Bass kernel for CustomGRUModel.

Reference computation (per batch row):
    gx = x @ W                       # [T, 3H] input projections
    per step t:
        gh_zr = h @ U[:, :2H]
        z = sigmoid(gxz + ghz + bz)
        r = sigmoid(gxr + ghr + br)
        n = tanh(gxn + (r*h) @ U[:, 2H:] + bn)
        h = z*h + (1-z)*n
    y = h_last @ Wd + bd

Sharding: data-parallel over batch, 32 rows per core on 8 cores. Weights
replicated. No collectives.

Per-core design (v5, fp16):
  - All matmul operands in fp16 (U, W, x, h, rh). fp32 would be lowered as
    TWO HW passes per matmul with a full-rate LDWEIGHTS each; 16-bit is one
    pass with 2x fast-weight-load, and at N=32 the weight loads fully hide
    behind the streaming (measured ~32ns per matmul). fp16 over bf16: same
    speed, 8x the mantissa precision (all values here are well in range).
  - Layout "features on partitions": hT [H=512, B=32] as TWO persistent
    SBUF tiles h_lo/h_hi [128, (k=2)x(b=32)] (split so consumers of each
    half release as soon as its updater finishes — Tile dependency
    tracking is per-tile). Gate matmuls keep U tiles [128,128] stationary,
    stream hT chunks (N=32), k-outer so each h/rh half is consumed as soon
    as it is ready; outputs land [3H, B] in PSUM so elementwise runs on
    full 128 partitions.
  - gx = x@W is computed in chunks of TC=4 steps directly INTO the
    recurrence PSUM banks; gate matmuls accumulate on top (start=False),
    so there are no per-step gx adds at all. Three per-gate PSUM tiles
    (z/r/n) of exactly one 2KB bank each, layout [128, (t=TC)(m=4)(b=32)]
    fp32, double-buffered (6 banks) + 1 bank x-transpose staging.
    PSUM "pending zero" discipline: the chronologically-first matmul into
    each bank uses start=True (marks the whole bank pending-zero: first
    touch overwrites, later touches accumulate), everything after uses
    start=False; the chronologically-last one uses stop=True.
  - Per-step critical chain: r-matmuls -> sigmoid(r) -> r*h (2 halves) ->
    n-matmuls (k-outer) -> tanh -> (1-z)*n -> h halves. z-path (sigmoid,
    1-z via VectorE, z*h on GpSimd) overlaps the n window. The x@W
    precompute for the next chunk is spliced in after the n-matmuls to
    fill TensorE during the tail.
"""

import os

import numpy as np

B, T, D, H = 256, 512, 256, 512
NCORES = 8
BL = B // NCORES  # 32 batch rows per core
TC = 4  # timestep chunk for the gx precompute (one PSUM bank per gate)
KH = H // 128  # 4 k-tiles over H
KD = D // 128  # 2 k-tiles over D
M3H = 3 * H // 128  # 12 m-tiles over 3H

_CACHE = {}


def _build(t_run, with_bias):
    from contextlib import ExitStack

    import concourse.bacc as bacc
    import concourse.bass as bass
    import concourse.tile as tile
    from concourse import masks, mybir

    dt = mybir.dt
    f32 = dt.float32
    f16 = dt.float16
    AF = mybir.ActivationFunctionType

    nchunk = t_run // TC

    nc = bacc.Bacc(
        "TRN2", target_bir_lowering=False, debug=False, num_devices=NCORES
    )
    x_d = nc.dram_tensor("x", [BL, T, D], f32, kind="ExternalInput")
    w_d = nc.dram_tensor("W", [D, 3 * H], f32, kind="ExternalInput")
    u_d = nc.dram_tensor("U", [H, 3 * H], f32, kind="ExternalInput")
    b_d = nc.dram_tensor("b", [3 * H], f32, kind="ExternalInput")
    wd_d = nc.dram_tensor("Wd", [H, 1], f32, kind="ExternalInput")
    bd_d = nc.dram_tensor("bd", [1], f32, kind="ExternalInput")
    y_d = nc.dram_tensor("y", [BL, 1], f32, kind="ExternalOutput")

    # chunked view of x: [chunk, tc, b, d]
    x_view = x_d.rearrange("b (c t) d -> c t b d", t=TC)

    with tile.TileContext(nc) as tc, ExitStack() as ctx:
        const = ctx.enter_context(tc.tile_pool(name="const", bufs=1))
        xin_pool = ctx.enter_context(tc.tile_pool(name="xin", bufs=4))
        xt_pool = ctx.enter_context(tc.tile_pool(name="xt", bufs=2))
        sb_pool = ctx.enter_context(tc.tile_pool(name="sb", bufs=3))
        # one PSUM pool per gate group; each tile is exactly one 2KB bank
        gz_psum = ctx.enter_context(
            tc.tile_pool(name="gzp", bufs=2, space=bass.MemorySpace.PSUM)
        )
        gr_psum = ctx.enter_context(
            tc.tile_pool(name="grp", bufs=2, space=bass.MemorySpace.PSUM)
        )
        gn_psum = ctx.enter_context(
            tc.tile_pool(name="gnp", bufs=2, space=bass.MemorySpace.PSUM)
        )
        xt_psum = ctx.enter_context(
            tc.tile_pool(name="xtp", bufs=2, space=bass.MemorySpace.PSUM)
        )

        # ---- constants (load fp32, cast to fp16 working copies) ----
        stage = const.tile([128, 3 * H], f32)
        u_sb = const.tile([128, KH, 3 * H], f16)
        for k in range(KH):
            nc.sync.dma_start(stage[:], u_d[k * 128 : (k + 1) * 128, :])
            nc.vector.tensor_copy(u_sb[:, k, :], stage[:])
        w_sb = const.tile([128, KD, 3 * H], f16)
        for k in range(KD):
            nc.sync.dma_start(stage[:], w_d[k * 128 : (k + 1) * 128, :])
            nc.vector.tensor_copy(w_sb[:, k, :], stage[:])

        b_sb = const.tile([128, M3H], f32)
        nc.sync.dma_start(b_sb[:], b_d.rearrange("(m p) -> p m", p=128))

        wd_stage = const.tile([128, KH], f32)
        nc.sync.dma_start(wd_stage[:], wd_d.rearrange("(k p) o -> p (k o)", p=128))
        wd_sb = const.tile([128, KH], f16)
        nc.vector.tensor_copy(wd_sb[:], wd_stage[:])
        bd_sb = const.tile([1, 1], f32)
        nc.sync.dma_start(bd_sb[0:1, :], bd_d.rearrange("(o u) -> o u", u=1))
        bd_f16 = const.tile([1, 1], f16)
        nc.vector.tensor_copy(bd_f16[0:1, :], bd_sb[0:1, :])
        ident = const.tile([128, 128], f32)
        masks.make_identity(nc, ident[:])
        ones_sb = const.tile([1, BL], f16)
        nc.gpsimd.memset(ones_sb[0:1, :], 1.0)

        # persistent hidden state hT, split into halves (k-chunks 0-1 / 2-3)
        h_half = [
            const.tile([128, 2 * BL], f16, name=f"h{i}") for i in range(2)
        ]
        nc.gpsimd.memset(h_half[0][:], 0.0)
        nc.gpsimd.memset(h_half[1][:], 0.0)

        def h_slice(k):
            return h_half[k // 2][:, (k % 2) * BL : (k % 2 + 1) * BL]

        warm_ps = xt_psum.tile([128, KD, 128], f32, name="warm", tag="xtp")
        nc.tensor.transpose(warm_ps[:, 0, :], ident[:], ident[:])

        gx_tiles = {}

        def make_units(c):
            """Emit-thunks for precomputing gx chunk c (TC steps) into PSUM.

            Per-gate tiles (z: m=0..3, r: 4..7, n: 8..11), layout
            [128, (t=TC)(mm=4)(b=32)] fp32 = one 2KB bank each, so the
            per-step gate slice [:, t, :, :] is 128 contiguous columns for
            the ScalarE reads. First matmul into each tile (mm=0, kd=0)
            uses start=True; all later ones start=False.
            """
            parts = (
                gz_psum.tile([128, TC, 4, BL], f32, name="gz", tag="gzp"),
                gr_psum.tile([128, TC, 4, BL], f32, name="gr", tag="grp"),
                gn_psum.tile([128, TC, 4, BL], f32, name="gn", tag="gnp"),
            )
            gx_tiles[c] = parts
            xin = xin_pool.tile([128, D], f32, name="xin", tag="xin")
            xin16 = xin_pool.tile([128, D], f16, name="xin16", tag="xin16")
            xt_sb = xt_pool.tile([128, KD, TC * BL], f16, name="xt", tag="xt")
            units = []

            def load():
                nc.sync.dma_start(xin[:], x_view[c])

            def cast():
                nc.vector.tensor_copy(xin16[:], xin[:])

            def tr(kd):
                # xbar DMA transpose (off TensorE; frees PE + the PSUM evict)
                nc.sync.dma_start_transpose(
                    xt_sb[:, kd, :], xin16[:, 128 * kd : 128 * (kd + 1)]
                )

            def mm(m):
                part = parts[m // 4]
                mm_i = m % 4
                for kd in range(KD):
                    nc.tensor.matmul(
                        part[:, :, mm_i, :],
                        w_sb[:, kd, m * 128 : (m + 1) * 128],
                        xt_sb[:, kd, :],
                        start=(kd == 0 and mm_i == 0),
                        stop=False,
                        skip_group_check=True,
                    )

            units.append(load)
            units.append(cast)
            for kd in range(KD):
                units.append(lambda kd=kd: tr(kd))
            for m in range(M3H):
                units.append(lambda m=m: mm(m))
            return units

        def emit_step(c, j, mid_units, dist):
            """One GRU step; gates accumulate into gx chunk tiles at t=j.

            The r-gate pre-activations for THIS step were already
            accumulated into gr during the previous step (distributed
            h-update: U_r@h = U_r@(z*h) + U_r@((1-z)*n); step 0 needs
            nothing since h0=0). This step emits the distributed r
            matmuls for the NEXT step into `dist` = (gr_tile, t, is_last)
            or None at the end of the sequence.

            mid_units: precompute emit-thunks spliced in after the n
            matmuls so they fill TensorE during the tanh tail.
            """
            gz_t, gr_t, gn_t = gx_tiles[c]
            last = j == TC - 1  # last step of chunk: emit stop=True per bank

            r_sb = sb_pool.tile([128, KH * BL], f16, name="r", tag="r")
            rh_half = [
                sb_pool.tile([128, 2 * BL], f16, name=f"rh{i}", tag=f"rh{i}")
                for i in range(2)
            ]
            z_sb = sb_pool.tile([128, KH * BL], f16, name="z", tag="z")
            zc_sb = sb_pool.tile([128, KH * BL], f16, name="zc", tag="zc")
            zh_sb = sb_pool.tile([128, KH * BL], f16, name="zh", tag="zh")
            n_sb = sb_pool.tile([128, KH * BL], f16, name="n", tag="n")
            zcn_sb = sb_pool.tile([128, KH * BL], f16, name="zcn", tag="zcn")

            # z gates (m=0..3), h-based, k-outer
            for k in range(KH):
                for mm_i in range(4):
                    nc.tensor.matmul(
                        gz_t[:, j, mm_i, :],
                        u_sb[:, k, mm_i * 128 : (mm_i + 1) * 128],
                        h_slice(k),
                        start=False,
                        stop=(last and k == KH - 1 and mm_i == 3),
                        skip_group_check=True,
                    )

            if with_bias:
                for i in range(4):
                    nc.scalar.activation(
                        r_sb[:, i * BL : (i + 1) * BL],
                        gr_t[:, j, i, :],
                        AF.Sigmoid,
                        bias=b_sb[:, 4 + i : 5 + i],
                    )
            else:
                nc.scalar.activation(
                    r_sb[:].rearrange("p (m b) -> p m b", m=4),
                    gr_t[:, j, :, :],
                    AF.Sigmoid,
                )
            for i in range(2):
                nc.vector.tensor_mul(
                    rh_half[i][:], r_sb[:, i * 2 * BL : (i + 1) * 2 * BL],
                    h_half[i][:],
                )

            if with_bias:
                for i in range(4):
                    nc.scalar.activation(
                        z_sb[:, i * BL : (i + 1) * BL],
                        gz_t[:, j, i, :],
                        AF.Sigmoid,
                        bias=b_sb[:, i : i + 1],
                    )
            else:
                nc.scalar.activation(
                    z_sb[:].rearrange("p (m b) -> p m b", m=4),
                    gz_t[:, j, :, :],
                    AF.Sigmoid,
                )
            nc.vector.tensor_scalar(
                zc_sb[:], z_sb[:], -1.0, 1.0,
                mybir.AluOpType.mult, mybir.AluOpType.add,
            )
            for i in range(2):
                nc.gpsimd.tensor_mul(
                    zh_sb[:, i * 2 * BL : (i + 1) * 2 * BL],
                    z_sb[:, i * 2 * BL : (i + 1) * 2 * BL],
                    h_half[i][:],
                )

            # n gates (m=8..11), k-outer: consume rh halves as they appear
            for k in range(KH):
                for mm_i in range(4):
                    nc.tensor.matmul(
                        gn_t[:, j, mm_i, :],
                        u_sb[:, k, (8 + mm_i) * 128 : (9 + mm_i) * 128],
                        rh_half[k // 2][:, (k % 2) * BL : (k % 2 + 1) * BL],
                        start=False,
                        stop=(last and k == KH - 1 and mm_i == 3),
                        skip_group_check=True,
                    )

            # distributed r matmuls for the NEXT step, z*h part (available
            # mid-window, runs during the tanh tail)
            if dist is not None:
                ngr_t, nj, nlast = dist
                for k in range(KH):
                    for mm_i in range(4):
                        nc.tensor.matmul(
                            ngr_t[:, nj, mm_i, :],
                            u_sb[:, k, (4 + mm_i) * 128 : (5 + mm_i) * 128],
                            zh_sb[:, k * BL : (k + 1) * BL],
                            start=False,
                            stop=False,
                            skip_group_check=True,
                        )

            # precompute filler: runs on TensorE during the tanh/update tail
            for u in mid_units:
                u()

            if with_bias:
                for i in range(4):
                    nc.scalar.activation(
                        n_sb[:, i * BL : (i + 1) * BL],
                        gn_t[:, j, i, :],
                        AF.Tanh,
                        bias=b_sb[:, 8 + i : 9 + i],
                    )
            else:
                nc.scalar.activation(
                    n_sb[:].rearrange("p (m b) -> p m b", m=4),
                    gn_t[:, j, :, :],
                    AF.Tanh,
                )
            nc.vector.tensor_mul(zcn_sb[:], zc_sb[:], n_sb[:])

            # distributed r matmuls for the NEXT step, (1-z)*n part — this
            # is the only piece of the next r pre-activation on the chain.
            if dist is not None:
                ngr_t, nj, nlast = dist
                for k in range(KH):
                    for mm_i in range(4):
                        nc.tensor.matmul(
                            ngr_t[:, nj, mm_i, :],
                            u_sb[:, k, (4 + mm_i) * 128 : (5 + mm_i) * 128],
                            zcn_sb[:, k * BL : (k + 1) * BL],
                            start=False,
                            stop=(nlast and k == KH - 1 and mm_i == 3),
                            skip_group_check=True,
                        )

            # h = z*h + (1-z)*n (off the critical chain now; feeds the
            # next step's z matmuls, r*h and z*h)
            for i in range(2):
                nc.vector.tensor_add(
                    h_half[i][:],
                    zh_sb[:, i * 2 * BL : (i + 1) * 2 * BL],
                    zcn_sb[:, i * 2 * BL : (i + 1) * 2 * BL],
                )

        # ---- main emission ----
        for u in make_units(0):
            u()
        for c in range(nchunk):
            pend = make_units(c + 1) if c + 1 < nchunk else []
            done = 0
            for j in range(TC):
                g = c * TC + j
                if g + 1 < t_run:
                    nc_, njj = divmod(g + 1, TC)
                    dist = (gx_tiles[nc_][1], njj, njj == TC - 1)
                else:
                    dist = None
                want = (len(pend) * (j + 1) + TC - 1) // TC
                mid = []
                while done < min(want, len(pend)):
                    mid.append(pend[done])
                    done += 1
                emit_step(c, j, mid, dist)
            while done < len(pend):
                pend[done]()
                done += 1

        # final dense head: y = h @ Wd + bd
        out_ps = xt_psum.tile([128, KD, 128], f32, name="outp", tag="xtp")
        for k in range(KH):
            nc.tensor.matmul(
                out_ps[0:BL, 0, 0:1],
                h_slice(k),
                wd_sb[:, k : k + 1],
                start=(k == 0),
                stop=False,
            )
        nc.tensor.matmul(
            out_ps[0:BL, 0, 0:1],
            ones_sb[0:1, :],
            bd_f16[0:1, :],
            start=False,
            stop=True,
        )
        y_sb = sb_pool.tile([BL, 1], f32, name="y", tag="y")
        nc.vector.tensor_copy(y_sb[:], out_ps[0:BL, 0, 0:1])
        nc.sync.dma_start(y_d[:], y_sb[:])

    nc.compile()
    return nc


def kernel(x, W, U, b, Wd, bd):
    from concourse.bass_utils import run_bass_kernel_spmd

    t_run = int(os.environ.get("GRU_T_RUN", T))

    x = np.ascontiguousarray(np.asarray(x, dtype=np.float32))
    W = np.ascontiguousarray(np.asarray(W, dtype=np.float32))
    U = np.ascontiguousarray(np.asarray(U, dtype=np.float32))
    b = np.ascontiguousarray(np.asarray(b, dtype=np.float32))
    Wd = np.ascontiguousarray(np.asarray(Wd, dtype=np.float32))
    bd = np.ascontiguousarray(np.asarray(bd, dtype=np.float32))

    with_bias = bool(np.any(b != 0.0))
    key = (t_run, with_bias)
    if key not in _CACHE:
        _CACHE[key] = _build(t_run, with_bias)
    nc = _CACHE[key]

    in_maps = [
        {
            "x": np.ascontiguousarray(x[i * BL : (i + 1) * BL]),
            "W": W,
            "U": U,
            "b": b,
            "Wd": Wd,
            "bd": bd,
        }
        for i in range(NCORES)
    ]
    res = run_bass_kernel_spmd(
        nc,
        in_maps,
        core_ids=list(range(NCORES)),
        trace=os.environ.get("GRU_TRACE", "0") == "1",
    )
    out = np.concatenate([r["y"] for r in res.results], axis=0)
    if res.exec_time_ns is not None:
        print(f"HW exec time: {res.exec_time_ns} ns")
    return out


# revision 19
# speedup vs baseline: 9.1438x; 1.0719x over previous
"""Trainium2 Bass kernel for CustomGRUModel.

Reference computation (per batch row):
    gx = x @ W                       # [T, 3H] input projections
    per step t:
        gh_zr = h @ U[:, :2H]
        z = sigmoid(gxz + ghz + bz)
        r = sigmoid(gxr + ghr + br)
        n = tanh(gxn + (r*h) @ U[:, 2H:] + bn)
        h = z*h + (1-z)*n
    y = h_last @ Wd + bd

Sharding: data-parallel over batch, 32 rows per core on 8 cores. Weights
replicated. No collectives.

Per-core design (v5, fp16):
  - All matmul operands in fp16 (U, W, x, h, rh). fp32 would be lowered as
    TWO HW passes per matmul with a full-rate LDWEIGHTS each; 16-bit is one
    pass with 2x fast-weight-load, and at N=32 the weight loads fully hide
    behind the streaming (measured ~32ns per matmul). fp16 over bf16: same
    speed, 8x the mantissa precision (all values here are well in range).
  - Layout "features on partitions": hT [H=512, B=32] as TWO persistent
    SBUF tiles h_lo/h_hi [128, (k=2)x(b=32)] (split so consumers of each
    half release as soon as its updater finishes — Tile dependency
    tracking is per-tile). Gate matmuls keep U tiles [128,128] stationary,
    stream hT chunks (N=32), k-outer so each h/rh half is consumed as soon
    as it is ready; outputs land [3H, B] in PSUM so elementwise runs on
    full 128 partitions.
  - gx = x@W is computed in chunks of TC=4 steps directly INTO the
    recurrence PSUM banks; gate matmuls accumulate on top (start=False),
    so there are no per-step gx adds at all. Three per-gate PSUM tiles
    (z/r/n) of exactly one 2KB bank each, layout [128, (t=TC)(m=4)(b=32)]
    fp32, double-buffered (6 banks) + 1 bank x-transpose staging.
    PSUM "pending zero" discipline: the chronologically-first matmul into
    each bank uses start=True (marks the whole bank pending-zero: first
    touch overwrites, later touches accumulate), everything after uses
    start=False; the chronologically-last one uses stop=True.
  - Per-step critical chain: r-matmuls -> sigmoid(r) -> r*h (2 halves) ->
    n-matmuls (k-outer) -> tanh -> (1-z)*n -> h halves. z-path (sigmoid,
    1-z via VectorE, z*h on GpSimd) overlaps the n window. The x@W
    precompute for the next chunk is spliced in after the n-matmuls to
    fill TensorE during the tail.
"""

import os

import numpy as np

B, T, D, H = 256, 512, 256, 512
NCORES = 8
BL = B // NCORES  # 32 batch rows per core
TC = 4  # timestep chunk for the gx precompute (one PSUM bank per gate)
KH = H // 128  # 4 k-tiles over H
KD = D // 128  # 2 k-tiles over D
M3H = 3 * H // 128  # 12 m-tiles over 3H

_CACHE = {}


def _build(t_run, with_bias):
    from contextlib import ExitStack

    import concourse.bacc as bacc
    import concourse.bass as bass
    import concourse.tile as tile
    from concourse import masks, mybir

    dt = mybir.dt
    f32 = dt.float32
    f16 = dt.float16
    AF = mybir.ActivationFunctionType

    nchunk = t_run // TC

    nc = bacc.Bacc(
        "TRN2", target_bir_lowering=False, debug=False, num_devices=NCORES
    )
    x_d = nc.dram_tensor("x", [BL, T, D], f32, kind="ExternalInput")
    w_d = nc.dram_tensor("W", [D, 3 * H], f32, kind="ExternalInput")
    u_d = nc.dram_tensor("U", [H, 3 * H], f32, kind="ExternalInput")
    b_d = nc.dram_tensor("b", [3 * H], f32, kind="ExternalInput")
    wd_d = nc.dram_tensor("Wd", [H, 1], f32, kind="ExternalInput")
    bd_d = nc.dram_tensor("bd", [1], f32, kind="ExternalInput")
    y_d = nc.dram_tensor("y", [BL, 1], f32, kind="ExternalOutput")

    # chunked view of x: [chunk, tc, b, d]
    x_view = x_d.rearrange("b (c t) d -> c t b d", t=TC)

    with tile.TileContext(nc) as tc, ExitStack() as ctx:
        const = ctx.enter_context(tc.tile_pool(name="const", bufs=1))
        xin_pool = ctx.enter_context(tc.tile_pool(name="xin", bufs=4))
        xt_pool = ctx.enter_context(tc.tile_pool(name="xt", bufs=2))
        sb_pool = ctx.enter_context(tc.tile_pool(name="sb", bufs=3))
        # one PSUM pool per gate group; each tile is exactly one 2KB bank
        gz_psum = ctx.enter_context(
            tc.tile_pool(name="gzp", bufs=2, space=bass.MemorySpace.PSUM)
        )
        gr_psum = ctx.enter_context(
            tc.tile_pool(name="grp", bufs=2, space=bass.MemorySpace.PSUM)
        )
        gn_psum = ctx.enter_context(
            tc.tile_pool(name="gnp", bufs=2, space=bass.MemorySpace.PSUM)
        )
        xt_psum = ctx.enter_context(
            tc.tile_pool(name="xtp", bufs=2, space=bass.MemorySpace.PSUM)
        )

        # ---- constants (load fp32, cast to fp16 working copies) ----
        stage = const.tile([128, 3 * H], f32)
        u_sb = const.tile([128, KH, 3 * H], f16)
        for k in range(KH):
            nc.sync.dma_start(stage[:], u_d[k * 128 : (k + 1) * 128, :])
            nc.vector.tensor_copy(u_sb[:, k, :], stage[:])
        w_sb = const.tile([128, KD, 3 * H], f16)
        for k in range(KD):
            nc.sync.dma_start(stage[:], w_d[k * 128 : (k + 1) * 128, :])
            nc.vector.tensor_copy(w_sb[:, k, :], stage[:])

        b_sb = const.tile([128, M3H], f32)
        nc.sync.dma_start(b_sb[:], b_d.rearrange("(m p) -> p m", p=128))

        wd_stage = const.tile([128, KH], f32)
        nc.sync.dma_start(wd_stage[:], wd_d.rearrange("(k p) o -> p (k o)", p=128))
        wd_sb = const.tile([128, KH], f16)
        nc.vector.tensor_copy(wd_sb[:], wd_stage[:])
        bd_sb = const.tile([1, 1], f32)
        nc.sync.dma_start(bd_sb[0:1, :], bd_d.rearrange("(o u) -> o u", u=1))
        bd_f16 = const.tile([1, 1], f16)
        nc.vector.tensor_copy(bd_f16[0:1, :], bd_sb[0:1, :])
        ident = const.tile([128, 128], f32)
        masks.make_identity(nc, ident[:])
        ones_sb = const.tile([1, BL], f16)
        nc.gpsimd.memset(ones_sb[0:1, :], 1.0)

        # persistent hidden state hT, split into halves (k-chunks 0-1 / 2-3)
        h_half = [
            const.tile([128, 2 * BL], f16, name=f"h{i}") for i in range(2)
        ]
        nc.gpsimd.memset(h_half[0][:], 0.0)
        nc.gpsimd.memset(h_half[1][:], 0.0)

        def h_slice(k):
            return h_half[k // 2][:, (k % 2) * BL : (k % 2 + 1) * BL]

        warm_ps = xt_psum.tile([128, KD, 128], f32, name="warm", tag="xtp")
        nc.tensor.transpose(warm_ps[:, 0, :], ident[:], ident[:])

        gx_tiles = {}

        def make_units(c):
            """Emit-thunks for precomputing gx chunk c (TC steps) into PSUM.

            Per-gate tiles (z: m=0..3, r: 4..7, n: 8..11), layout
            [128, (t=TC)(mm=4)(b=32)] fp32 = one 2KB bank each, so the
            per-step gate slice [:, t, :, :] is 128 contiguous columns for
            the ScalarE reads. First matmul into each tile (mm=0, kd=0)
            uses start=True; all later ones start=False.
            """
            parts = (
                gz_psum.tile([128, TC, 4, BL], f32, name="gz", tag="gzp"),
                gr_psum.tile([128, TC, 4, BL], f32, name="gr", tag="grp"),
                gn_psum.tile([128, TC, 4, BL], f32, name="gn", tag="gnp"),
            )
            gx_tiles[c] = parts
            xin = xin_pool.tile([128, D], f32, name="xin", tag="xin")
            xt_ps = xt_psum.tile([128, KD, TC * BL], f32, name="xtp", tag="xtp")
            xt_sb = xt_pool.tile([128, KD, TC * BL], f16, name="xt", tag="xt")
            units = []

            def load():
                nc.sync.dma_start(xin[:], x_view[c])

            def tr(kd):
                nc.tensor.transpose(
                    xt_ps[:, kd, :], xin[:, 128 * kd : 128 * (kd + 1)], ident[:]
                )

            def evict(kd):
                nc.vector.tensor_copy(xt_sb[:, kd, :], xt_ps[:, kd, :])

            def mm(m):
                part = parts[m // 4]
                mm_i = m % 4
                for kd in range(KD):
                    nc.tensor.matmul(
                        part[:, :, mm_i, :],
                        w_sb[:, kd, m * 128 : (m + 1) * 128],
                        xt_sb[:, kd, :],
                        start=(kd == 0 and mm_i == 0),
                        stop=False,
                        skip_group_check=True,
                    )

            units.append(load)
            for kd in range(KD):
                units.append(lambda kd=kd: tr(kd))
            for kd in range(KD):
                units.append(lambda kd=kd: evict(kd))
            for m in range(M3H):
                units.append(lambda m=m: mm(m))
            return units

        def emit_step(c, j, mid_units, dist):
            """One GRU step; gates accumulate into gx chunk tiles at t=j.

            The r-gate pre-activations for THIS step were already
            accumulated into gr during the previous step (distributed
            h-update: U_r@h = U_r@(z*h) + U_r@((1-z)*n); step 0 needs
            nothing since h0=0). This step emits the distributed r
            matmuls for the NEXT step into `dist` = (gr_tile, t, is_last)
            or None at the end of the sequence.

            mid_units: precompute emit-thunks spliced in after the n
            matmuls so they fill TensorE during the tanh tail.
            """
            gz_t, gr_t, gn_t = gx_tiles[c]
            last = j == TC - 1  # last step of chunk: emit stop=True per bank

            r_sb = sb_pool.tile([128, KH * BL], f16, name="r", tag="r")
            rh_half = [
                sb_pool.tile([128, 2 * BL], f16, name=f"rh{i}", tag=f"rh{i}")
                for i in range(2)
            ]
            z_sb = sb_pool.tile([128, KH * BL], f16, name="z", tag="z")
            zc_sb = sb_pool.tile([128, KH * BL], f16, name="zc", tag="zc")
            zh_sb = sb_pool.tile([128, KH * BL], f16, name="zh", tag="zh")
            n_sb = sb_pool.tile([128, KH * BL], f16, name="n", tag="n")
            zcn_sb = sb_pool.tile([128, KH * BL], f16, name="zcn", tag="zcn")

            # z gates (m=0..3), h-based, k-outer
            for k in range(KH):
                for mm_i in range(4):
                    nc.tensor.matmul(
                        gz_t[:, j, mm_i, :],
                        u_sb[:, k, mm_i * 128 : (mm_i + 1) * 128],
                        h_slice(k),
                        start=False,
                        stop=(last and k == KH - 1 and mm_i == 3),
                        skip_group_check=True,
                    )

            if with_bias:
                for i in range(4):
                    nc.scalar.activation(
                        r_sb[:, i * BL : (i + 1) * BL],
                        gr_t[:, j, i, :],
                        AF.Sigmoid,
                        bias=b_sb[:, 4 + i : 5 + i],
                    )
            else:
                nc.scalar.activation(
                    r_sb[:].rearrange("p (m b) -> p m b", m=4),
                    gr_t[:, j, :, :],
                    AF.Sigmoid,
                )
            for i in range(2):
                nc.vector.tensor_mul(
                    rh_half[i][:], r_sb[:, i * 2 * BL : (i + 1) * 2 * BL],
                    h_half[i][:],
                )

            if with_bias:
                for i in range(4):
                    nc.scalar.activation(
                        z_sb[:, i * BL : (i + 1) * BL],
                        gz_t[:, j, i, :],
                        AF.Sigmoid,
                        bias=b_sb[:, i : i + 1],
                    )
            else:
                nc.scalar.activation(
                    z_sb[:].rearrange("p (m b) -> p m b", m=4),
                    gz_t[:, j, :, :],
                    AF.Sigmoid,
                )
            nc.vector.tensor_scalar(
                zc_sb[:], z_sb[:], -1.0, 1.0,
                mybir.AluOpType.mult, mybir.AluOpType.add,
            )
            for i in range(2):
                nc.gpsimd.tensor_mul(
                    zh_sb[:, i * 2 * BL : (i + 1) * 2 * BL],
                    z_sb[:, i * 2 * BL : (i + 1) * 2 * BL],
                    h_half[i][:],
                )

            # n gates (m=8..11), k-outer: consume rh halves as they appear
            for k in range(KH):
                for mm_i in range(4):
                    nc.tensor.matmul(
                        gn_t[:, j, mm_i, :],
                        u_sb[:, k, (8 + mm_i) * 128 : (9 + mm_i) * 128],
                        rh_half[k // 2][:, (k % 2) * BL : (k % 2 + 1) * BL],
                        start=False,
                        stop=(last and k == KH - 1 and mm_i == 3),
                        skip_group_check=True,
                    )

            # distributed r matmuls for the NEXT step, z*h part (available
            # mid-window, runs during the tanh tail)
            if dist is not None:
                ngr_t, nj, nlast = dist
                for k in range(KH):
                    for mm_i in range(4):
                        nc.tensor.matmul(
                            ngr_t[:, nj, mm_i, :],
                            u_sb[:, k, (4 + mm_i) * 128 : (5 + mm_i) * 128],
                            zh_sb[:, k * BL : (k + 1) * BL],
                            start=False,
                            stop=False,
                            skip_group_check=True,
                        )

            # precompute filler: runs on TensorE during the tanh/update tail
            for u in mid_units:
                u()

            if with_bias:
                for i in range(4):
                    nc.scalar.activation(
                        n_sb[:, i * BL : (i + 1) * BL],
                        gn_t[:, j, i, :],
                        AF.Tanh,
                        bias=b_sb[:, 8 + i : 9 + i],
                    )
            else:
                nc.scalar.activation(
                    n_sb[:].rearrange("p (m b) -> p m b", m=4),
                    gn_t[:, j, :, :],
                    AF.Tanh,
                )
            nc.vector.tensor_mul(zcn_sb[:], zc_sb[:], n_sb[:])

            # distributed r matmuls for the NEXT step, (1-z)*n part — this
            # is the only piece of the next r pre-activation on the chain.
            if dist is not None:
                ngr_t, nj, nlast = dist
                for k in range(KH):
                    for mm_i in range(4):
                        nc.tensor.matmul(
                            ngr_t[:, nj, mm_i, :],
                            u_sb[:, k, (4 + mm_i) * 128 : (5 + mm_i) * 128],
                            zcn_sb[:, k * BL : (k + 1) * BL],
                            start=False,
                            stop=(nlast and k == KH - 1 and mm_i == 3),
                            skip_group_check=True,
                        )

            # h = z*h + (1-z)*n (off the critical chain now; feeds the
            # next step's z matmuls, r*h and z*h)
            for i in range(2):
                nc.vector.tensor_add(
                    h_half[i][:],
                    zh_sb[:, i * 2 * BL : (i + 1) * 2 * BL],
                    zcn_sb[:, i * 2 * BL : (i + 1) * 2 * BL],
                )

        # ---- main emission ----
        for u in make_units(0):
            u()
        for c in range(nchunk):
            pend = make_units(c + 1) if c + 1 < nchunk else []
            done = 0
            for j in range(TC):
                g = c * TC + j
                if g + 1 < t_run:
                    nc_, njj = divmod(g + 1, TC)
                    dist = (gx_tiles[nc_][1], njj, njj == TC - 1)
                else:
                    dist = None
                want = (len(pend) * (j + 1) + TC - 1) // TC
                mid = []
                while done < min(want, len(pend)):
                    mid.append(pend[done])
                    done += 1
                emit_step(c, j, mid, dist)
            while done < len(pend):
                pend[done]()
                done += 1

        # final dense head: y = h @ Wd + bd
        out_ps = xt_psum.tile([128, KD, 128], f32, name="outp", tag="xtp")
        for k in range(KH):
            nc.tensor.matmul(
                out_ps[0:BL, 0, 0:1],
                h_slice(k),
                wd_sb[:, k : k + 1],
                start=(k == 0),
                stop=False,
            )
        nc.tensor.matmul(
            out_ps[0:BL, 0, 0:1],
            ones_sb[0:1, :],
            bd_f16[0:1, :],
            start=False,
            stop=True,
        )
        y_sb = sb_pool.tile([BL, 1], f32, name="y", tag="y")
        nc.vector.tensor_copy(y_sb[:], out_ps[0:BL, 0, 0:1])
        nc.sync.dma_start(y_d[:], y_sb[:])

    nc.compile()
    return nc


def kernel(x, W, U, b, Wd, bd):
    from concourse.bass_utils import run_bass_kernel_spmd

    t_run = int(os.environ.get("GRU_T_RUN", T))

    x = np.ascontiguousarray(np.asarray(x, dtype=np.float32))
    W = np.ascontiguousarray(np.asarray(W, dtype=np.float32))
    U = np.ascontiguousarray(np.asarray(U, dtype=np.float32))
    b = np.ascontiguousarray(np.asarray(b, dtype=np.float32))
    Wd = np.ascontiguousarray(np.asarray(Wd, dtype=np.float32))
    bd = np.ascontiguousarray(np.asarray(bd, dtype=np.float32))

    with_bias = bool(np.any(b != 0.0))
    key = (t_run, with_bias)
    if key not in _CACHE:
        _CACHE[key] = _build(t_run, with_bias)
    nc = _CACHE[key]

    in_maps = [
        {
            "x": np.ascontiguousarray(x[i * BL : (i + 1) * BL]),
            "W": W,
            "U": U,
            "b": b,
            "Wd": Wd,
            "bd": bd,
        }
        for i in range(NCORES)
    ]
    res = run_bass_kernel_spmd(
        nc,
        in_maps,
        core_ids=list(range(NCORES)),
        trace=os.environ.get("GRU_TRACE", "0") == "1",
    )
    out = np.concatenate([r["y"] for r in res.results], axis=0)
    if res.exec_time_ns is not None:
        print(f"HW exec time: {res.exec_time_ns} ns")
    return out
